# revision 11
# baseline (speedup 1.0000x reference)
"""nn_Attention_30511447671564 — head-mixing attention block on 8 trn2 cores.

Math (B=64, T=64, C=4096, H=64, hd=64, rank=1; n_tok = B*T = 4096):
  qkv = x @ W_atten^T + b_atten                       (tok, 3C)
  per-token attention ACROSS the 64 heads (HxH), causal over head index
  out = y @ W_proj^T + b_proj                         (tok, C)

Distribution (Megatron-ish):
  - QKV GEMM column-parallel: core c owns heads [8c, 8c+8) of q/k/v
    (1536 rows of W_atten), computes them for ALL 4096 tokens.
  - x is fed as per-core token slabs, transposed+cast to bf16 on device,
    then AllGathered so every core has x^T (c-major) for all tokens.
  - AllToAll redistributes qkv^T from feature-sharded to token-sharded.
  - Attention + proj are data-parallel over the core's 512 tokens.
  - proj uses a permuted W_proj^T ("WPT") built cooperatively (each core
    transposes 512 permuted columns) and AllGathered during the QKV GEMM.

All on-chip transposes go through the PE (transpose-mode matmul with an
identity stationary) + DVE eviction — the DMA xbar path costs ~1.2us of
HWDGE issue time per 128x128 tile, which serializes.

All matmuls run in bf16 (fp32 accumulate in PSUM); rel-err budget 2e-2.
"""

import numpy as np
import ml_dtypes

B, T, C = 64, 64, 4096
H, HD = 64, 64
NCORES = 8
NTOK = B * T            # 4096
TPC = NTOK // NCORES    # 512 tokens per core
OS = 3 * C // NCORES    # 1536 qkv feature rows per core
NCHUNK = 2              # attention processed in chunks of tokens

_BF16 = ml_dtypes.bfloat16


def _build_program(tpc=TPC):
    import concourse.bass as bass
    import concourse.mybir as mybir
    from concourse import tile, bacc

    bf16 = mybir.dt.bfloat16
    f32 = mybir.dt.float32
    AF = mybir.ActivationFunctionType
    ALU = mybir.AluOpType
    ALL = [list(range(NCORES))]
    cht = tpc // NCHUNK

    nc = bacc.Bacc("TRN2", target_bir_lowering=False, debug=False,
                   num_devices=NCORES)

    # ---------------- external I/O ----------------
    x_sl = nc.dram_tensor("x_sl", [tpc, C], f32, kind="ExternalInput").ap()
    wa_sl = nc.dram_tensor("wa_sl", [OS, C], f32, kind="ExternalInput").ap()
    ba_sl = nc.dram_tensor("ba_sl", [OS], f32, kind="ExternalInput").ap()
    wp_sl = nc.dram_tensor("wp_sl", [C, 512], f32, kind="ExternalInput").ap()
    bp = nc.dram_tensor("bp", [C], f32, kind="ExternalInput").ap()
    mask8 = nc.dram_tensor("mask8", [H, 512], bf16, kind="ExternalInput").ap()
    ident = nc.dram_tensor("ident", [128, 128], bf16, kind="ExternalInput").ap()
    out_ext = nc.dram_tensor("out", [tpc, C], f32, kind="ExternalOutput").ap()

    # ---------------- internal DRAM ----------------
    xg_in = nc.dram_tensor("xg_in", [C, tpc], bf16).ap()
    xt_ag = [nc.dram_tensor(f"xt_ag{h}", [NCORES * C // 2, tpc], bf16,
                            addr_space="Shared").ap() for h in range(2)]
    wp_my = nc.dram_tensor("wp_my", [512, C], bf16).ap()
    wpt_ag = nc.dram_tensor("wpt_ag", [C, C], bf16, addr_space="Shared").ap()
    qkv_send = [nc.dram_tensor(f"qkv_send{i}", [NCORES, 512, tpc], bf16).ap()
                for i in range(3)]
    qkv_recv = [nc.dram_tensor(f"qkv_recv{i}", [NCORES, 512, tpc], bf16).ap()
                for i in range(3)]

    CT = C // 128        # 32 contraction tiles
    OT = OS // 128       # 12 output tiles for qkv slice
    PT = C // 128        # 32 output tiles for proj

    with tile.TileContext(nc) as tc:
        with tc.tile_pool(name="const", bufs=1) as constp:
            ba_t = constp.tile([128, OT], f32)
            nc.sync.dma_start(out=ba_t[:], in_=ba_sl.rearrange("(o p) -> p o", p=128))
            bp_t = constp.tile([128, PT], f32)
            nc.sync.dma_start(out=bp_t[:], in_=bp.rearrange("(o p) -> p o", p=128))
            mask_t = constp.tile([H, 512], bf16)
            nc.sync.dma_start(out=mask_t[:], in_=mask8[:])
            id_t = constp.tile([128, 128], bf16)
            nc.sync.dma_start(out=id_t[:], in_=ident[:])

            def pe_transpose(trps, dst_ap, src_ap):
                """128x128 bf16 transpose: PE transpose-mode + DVE evict."""
                tp = trps.tile([128, 128], bf16, tag="trp")
                nc.tensor.transpose(tp[:], src_ap, id_t[:])
                nc.vector.tensor_copy(dst_ap, tp[:])

            # ======== prep + QKV GEMM (WaT resident in SBUF) ========
            with tc.tile_pool(name="wat", bufs=1) as watp:
                wat = watp.tile([128, CT * OT * 128], bf16)

                with (
                    tc.tile_pool(name="xprep", bufs=2) as xprep,
                    tc.tile_pool(name="xts", bufs=1) as xtsp,
                    tc.tile_pool(name="wpprep", bufs=2) as wpprep,
                    tc.tile_pool(name="wpst", bufs=1) as wpstp,
                    tc.tile_pool(name="trps", bufs=4, space="PSUM") as trps,
                ):
                    # P0: x slab -> bf16 -> transpose -> AllGather (2 halves)
                    xgv = xg_in.rearrange("(c p) t -> p c t", p=128)
                    xts = xtsp.tile([128, CT, tpc], bf16)
                    for half in range(2):
                        c0 = half * (CT // 2)
                        for tt in range(tpc // 128):
                            xb = xprep.tile([128, C // 2], bf16, tag="xb")
                            nc.gpsimd.dma_start(
                                out=xb[:],
                                in_=x_sl[tt * 128:(tt + 1) * 128,
                                         c0 * 128:(c0 + CT // 2) * 128])
                            for k in range(CT // 2):
                                pe_transpose(
                                    trps,
                                    xts[:, c0 + k, tt * 128:(tt + 1) * 128],
                                    xb[:, k * 128:(k + 1) * 128])
                        nc.sync.dma_start(
                            out=xgv[:, c0:c0 + CT // 2, :],
                            in_=xts[:, c0:c0 + CT // 2, :])
                        nc.gpsimd.collective_compute(
                            "AllGather", ALU.bypass, replica_groups=ALL,
                            ins=[xg_in[c0 * 128:(c0 + CT // 2) * 128, :].opt()],
                            outs=[xt_ag[half].opt()],
                        )

                    # P1: Wa slice -> bf16 -> transpose into resident WaT
                    for ot in range(OT):
                        wab = xprep.tile([128, C], bf16, tag="xb")
                        nc.gpsimd.dma_start(
                            out=wab[:], in_=wa_sl[ot * 128:(ot + 1) * 128, :])
                        for ct in range(CT):
                            pe_transpose(
                                trps,
                                wat[:, (ct * OT + ot) * 128:
                                    (ct * OT + ot + 1) * 128],
                                wab[:, ct * 128:(ct + 1) * 128])

                    # P0b: Wp slice -> WPT slab -> AllGather
                    wpst = wpstp.tile([128, 4, C], bf16)
                    for ot in range(PT):
                        wpb = wpprep.tile([128, 512], bf16, tag="wpb")
                        nc.gpsimd.dma_start(
                            out=wpb[:], in_=wp_sl[ot * 128:(ot + 1) * 128, :])
                        for jt in range(4):
                            pe_transpose(trps,
                                         wpst[:, jt, ot * 128:(ot + 1) * 128],
                                         wpb[:, jt * 128:(jt + 1) * 128])
                    nc.sync.dma_start(
                        out=wp_my.rearrange("(j p) o -> p j o", p=128), in_=wpst[:]
                    )
                    nc.gpsimd.collective_compute(
                        "AllGather", ALU.bypass, replica_groups=ALL,
                        ins=[wp_my.opt()], outs=[wpt_ag.opt()],
                    )

                # P2: QKV GEMM (column-parallel over features, all tokens)
                with (
                    tc.tile_pool(name="xs", bufs=2) as xsp,
                    tc.tile_pool(name="qkvps", bufs=2, space="PSUM") as qkvps,
                    tc.tile_pool(name="qkvev", bufs=3) as qkvev,
                ):
                    for g in range(NCORES):
                        xs = xsp.tile([128, CT, tpc], bf16)
                        for half in range(2):
                            c0 = half * (CT // 2)
                            nc.sync.dma_start(
                                out=xs[:, c0:c0 + CT // 2, :],
                                in_=xt_ag[half][g * C // 2:(g + 1) * C // 2,
                                                :].rearrange(
                                    "(c p) t -> p c t", p=128),
                            )
                        for ot in range(OT):
                            ps = qkvps.tile([128, tpc], f32)
                            for ct in range(CT):
                                nc.tensor.matmul(
                                    ps[:],
                                    wat[:, (ct * OT + ot) * 128:
                                        (ct * OT + ot + 1) * 128],
                                    xs[:, ct, :],
                                    start=(ct == 0), stop=(ct == CT - 1),
                                )
                            ev = qkvev.tile([128, tpc], bf16)
                            nc.vector.tensor_scalar(
                                ev[:], ps[:], ba_t[:, ot:ot + 1], None, ALU.add)
                            nc.sync.dma_start(
                                out=qkv_send[ot // 4][g, (ot % 4) * 128:
                                                      (ot % 4 + 1) * 128, :],
                                in_=ev[:])

            # ======== P3: AllToAll to token-sharded (q, k, v separately) ====
            for i in range(3):
                nc.gpsimd.collective_compute(
                    "AllToAll", ALU.bypass, replica_groups=ALL,
                    ins=[qkv_send[i].opt()], outs=[qkv_recv[i].opt()],
                )

            with tc.tile_pool(name="yt", bufs=1) as ytp:
                # y^T in proj-contraction layout:
                #  partitions p<64 : (i=p)  y[t, i*64 + 2e]   at free e*tpc+t
                #  partitions p>=64: (i=p-64) y[t, i*64+2e+1] at free e*tpc+t
                yt = ytp.tile([128, 32 * tpc], bf16)
                ytv = yt.rearrange("p (e t) -> p e t", t=tpc)

                # ======== P4+P5: attention, NCHUNK chunks of tokens ========
                for ch in range(NCHUNK):
                    t0 = ch * cht
                    with (
                        tc.tile_pool(name="qkt", bufs=1) as qktp,
                        tc.tile_pool(name="vt", bufs=1) as vtp,
                        tc.tile_pool(name="att_sb", bufs=4) as attsb,
                        tc.tile_pool(name="att_ps", bufs=3, space="PSUM") as attps,
                        tc.tile_pool(name="u_ps", bufs=3, space="PSUM") as ups,
                        tc.tile_pool(name="r_sb", bufs=2) as rsb,
                    ):
                        qt = qktp.tile([H, H * cht], bf16, tag="qt")
                        kt = qktp.tile([H, H * cht], bf16, tag="kt")
                        vte = vtp.tile([H, 33 * cht], bf16, tag="vte")
                        vto = vtp.tile([H, 33 * cht], bf16, tag="vto")
                        for j in range(NCORES):
                            qv = qkv_recv[0][j].rearrange(
                                "(h d) t -> d h t", d=64)
                            kv = qkv_recv[1][j].rearrange(
                                "(h d) t -> d h t", d=64)
                            vv = qkv_recv[2][j].rearrange(
                                "(h d) t -> h d t", d=64)
                            nc.sync.dma_start(
                                out=qt.rearrange("d (h t) -> d h t", t=cht)
                                      [:, 8 * j:8 * (j + 1), :],
                                in_=qv[:, :, t0:t0 + cht])
                            nc.sync.dma_start(
                                out=kt.rearrange("d (h t) -> d h t", t=cht)
                                      [:, 8 * j:8 * (j + 1), :],
                                in_=kv[:, :, t0:t0 + cht])
                            nc.sync.dma_start(
                                out=vte[8 * j:8 * (j + 1), 0:32 * cht],
                                in_=vv[:, 0:64:2, t0:t0 + cht])
                            nc.sync.dma_start(
                                out=vto[8 * j:8 * (j + 1), 0:32 * cht],
                                in_=vv[:, 1:64:2, t0:t0 + cht])
                        nc.vector.memset(vte[:, 32 * cht:33 * cht], 1.0)
                        nc.vector.memset(vto[:, 32 * cht:33 * cht], 1.0)

                        # HAM warm-keepers: trivial matmuls, each gated on one
                        # shuffle DMA landing, keep PE busy-windows alive
                        # across the AllToAll/shuffle gap.
                        with tc.tile_pool(name="fill_ps", bufs=2,
                                          space="PSUM") as fillps:
                            for j in range(NCORES):
                                for src_t in (qt, kt):
                                    fp = fillps.tile([64, 64], f32, tag="f")
                                    nc.tensor.matmul(
                                        fp[:],
                                        src_t[:, (8 * j) * cht:
                                              (8 * j) * cht + 64],
                                        src_t[:, (8 * j) * cht:
                                              (8 * j) * cht + 64],
                                        start=True, stop=True)

                        def emit_s(tb):
                            psS = attps.tile([H, 512], f32, tag="psS")
                            for tt in range(8):
                                tm = tb * 8 + tt
                                nc.tensor.matmul(
                                    psS[:, tt * 64:(tt + 1) * 64],
                                    kt[:, tm:tm + (H - 1) * cht + 1:cht],
                                    qt[:, tm:tm + (H - 1) * cht + 1:cht],
                                    start=True, stop=True,
                                )
                            e8 = attsb.tile([H, 512], bf16, tag="e8")
                            nc.scalar.activation(e8[:], psS[:], AF.Exp,
                                                 scale=0.125)
                            e8m = attsb.tile([H, 512], bf16, tag="e8m")
                            nc.vector.tensor_tensor(e8m[:], e8[:], mask_t[:],
                                                    ALU.mult)
                            return e8m

                        def emit_u(tb, e8m):
                            psU = ups.tile([128, 8 * 33], f32, tag="psU")
                            for tt in range(8):
                                tm = tb * 8 + tt
                                nc.tensor.matmul(
                                    psU[0:64, tt * 33:(tt + 1) * 33],
                                    e8m[:, tt * 64:(tt + 1) * 64],
                                    vte[:, tm:tm + 32 * cht + 1:cht],
                                    start=True, stop=True,
                                )
                                nc.tensor.matmul(
                                    psU[64:128, tt * 33:(tt + 1) * 33],
                                    e8m[:, tt * 64:(tt + 1) * 64],
                                    vto[:, tm:tm + 32 * cht + 1:cht],
                                    start=True, stop=True,
                                )
                            r8 = rsb.tile([128, 8], f32)
                            nc.vector.reciprocal(
                                r8[:], psU[:, 32:32 + 7 * 33 + 1:33])
                            psUv = psU.rearrange("p (t c) -> p c t", c=33)
                            tg0 = t0 + tb * 8
                            for lo, hi in ((0, 64), (64, 128)):
                                nc.vector.tensor_tensor(
                                    ytv[lo:hi, :, tg0:tg0 + 8],
                                    psUv[lo:hi, 0:32, :],
                                    r8[lo:hi, None, :].broadcast_to(
                                        [hi - lo, 32, 8]),
                                    ALU.mult)

                        pend = None
                        for tb in range(cht // 8):
                            e8m = emit_s(tb)
                            if pend is not None:
                                emit_u(tb - 1, pend)
                            pend = e8m
                        emit_u(cht // 8 - 1, pend)

                # ======== P6: proj GEMM + transpose eviction ========
                with (
                    tc.tile_pool(name="wpt_in", bufs=32) as wptin,
                    tc.tile_pool(name="projps", bufs=2, space="PSUM") as projps,
                    tc.tile_pool(name="ptrps", bufs=4, space="PSUM") as ptrps,
                    tc.tile_pool(name="projev", bufs=2) as projev,
                    tc.tile_pool(name="projtr", bufs=4) as projtr,
                ):
                    for ot in range(PT):
                        ps = projps.tile([128, tpc], f32)
                        for e4 in range(8):
                            wt4 = wptin.tile([128, 4, 128], bf16)
                            nc.sync.dma_start(
                                out=wt4[:],
                                in_=wpt_ag[e4 * 512:(e4 + 1) * 512,
                                           ot * 128:(ot + 1) * 128].rearrange(
                                               "(e p) o -> p e o", p=128))
                            for k in range(4):
                                e = e4 * 4 + k
                                nc.tensor.matmul(
                                    ps[:], wt4[:, k, :],
                                    yt[:, e * tpc:(e + 1) * tpc],
                                    start=(e == 0), stop=(e == 31),
                                )
                        ev = projev.tile([128, tpc], bf16)
                        nc.vector.tensor_scalar(
                            ev[:], ps[:], bp_t[:, ot:ot + 1], None, ALU.add)
                        for tt in range(tpc // 128):
                            tp2 = ptrps.tile([128, 128], bf16, tag="ptr")
                            nc.tensor.transpose(
                                tp2[:], ev[:, tt * 128:(tt + 1) * 128], id_t[:])
                            tr = projtr.tile([128, 128], bf16)
                            nc.vector.tensor_copy(tr[:], tp2[:])
                            nc.gpsimd.dma_start(
                                out=out_ext[tt * 128:(tt + 1) * 128,
                                            ot * 128:(ot + 1) * 128],
                                in_=tr[:])

    nc.compile()
    return nc


_CACHED_NC = None


def _get_nc():
    global _CACHED_NC
    if _CACHED_NC is None:
        _CACHED_NC = _build_program()
    return _CACHED_NC


def host_inputs(x, W_atten, b_atten, W_proj, b_proj, tpc=TPC):
    """Slice/permute the full inputs into per-core input maps."""
    x = np.asarray(x, dtype=np.float32).reshape(-1, C)
    W_atten = np.asarray(W_atten, dtype=np.float32)
    b_atten = np.asarray(b_atten, dtype=np.float32)
    W_proj = np.asarray(W_proj, dtype=np.float32)
    b_proj = np.asarray(b_proj, dtype=np.float32)

    # causal mask over head index, replicated for 8-token exp batches
    m = np.tril(np.ones((H, H), dtype=np.float32)).T  # m[j,i] = 1 if j <= i
    mask8 = np.tile(m, (1, 8)).astype(_BF16)
    identity = np.eye(128, dtype=np.float32).astype(_BF16)

    in_maps = []
    for c in range(NCORES):
        r0 = c * 512
        wa = np.concatenate([
            W_atten[r0:r0 + 512],
            W_atten[C + r0:C + r0 + 512],
            W_atten[2 * C + r0:2 * C + r0 + 512],
        ], axis=0)
        ba = np.concatenate([
            b_atten[r0:r0 + 512],
            b_atten[C + r0:C + r0 + 512],
            b_atten[2 * C + r0:2 * C + r0 + 512],
        ], axis=0)
        js = np.arange(512)
        k = js // 128
        p = js % 128
        i = p % 64
        d = 8 * c + 2 * k + (p >= 64)
        cols = i * 64 + d
        wp = np.ascontiguousarray(W_proj[:, cols])
        in_maps.append({
            "x_sl": np.ascontiguousarray(x[c * tpc:(c + 1) * tpc]),
            "wa_sl": np.ascontiguousarray(wa),
            "ba_sl": np.ascontiguousarray(ba),
            "wp_sl": wp,
            "bp": b_proj,
            "mask8": mask8,
            "ident": identity,
        })
    return in_maps


def kernel(x, W_atten, b_atten, W_proj, b_proj):
    from concourse.bass_utils import run_bass_kernel_spmd

    nc = _get_nc()
    in_maps = host_inputs(x, W_atten, b_atten, W_proj, b_proj)
    res = run_bass_kernel_spmd(nc, in_maps, list(range(NCORES)))
    out = np.concatenate([res.results[c]["out"] for c in range(NCORES)], axis=0)
    return out.reshape(B, T, C)


# revision 14
# speedup vs baseline: 1.0265x; 1.0265x over previous
"""nn_Attention_30511447671564 — head-mixing attention block on 8 trn2 cores.

Math (B=64, T=64, C=4096, H=64, hd=64, rank=1; n_tok = B*T = 4096):
  qkv = x @ W_atten^T + b_atten                       (tok, 3C)
  per-token attention ACROSS the 64 heads (HxH), causal over head index
  out = y @ W_proj^T + b_proj                         (tok, C)

Distribution (Megatron-ish):
  - QKV GEMM column-parallel: core c owns heads [8c, 8c+8) of q/k/v
    (1536 rows of W_atten), computes them for ALL 4096 tokens.
  - x is fed as per-core token slabs, transposed+cast to bf16 on device,
    then AllGathered so every core has x^T (c-major) for all tokens.
  - AllToAll redistributes qkv^T from feature-sharded to token-sharded.
  - Attention + proj are data-parallel over the core's 512 tokens.
  - proj uses a permuted W_proj^T ("WPT") built cooperatively (each core
    transposes 512 permuted columns) and AllGathered during the QKV GEMM.

All on-chip transposes go through the PE (transpose-mode matmul with an
identity stationary) + DVE eviction — the DMA xbar path costs ~1.2us of
HWDGE issue time per 128x128 tile, which serializes.

All matmuls run in bf16 (fp32 accumulate in PSUM); rel-err budget 2e-2.
"""

import numpy as np
import ml_dtypes

B, T, C = 64, 64, 4096
H, HD = 64, 64
NCORES = 8
NTOK = B * T            # 4096
TPC = NTOK // NCORES    # 512 tokens per core
OS = 3 * C // NCORES    # 1536 qkv feature rows per core
NCHUNK = 2              # attention processed in chunks of tokens

_BF16 = ml_dtypes.bfloat16


def _build_program(tpc=TPC):
    import concourse.bass as bass
    import concourse.mybir as mybir
    from concourse import tile, bacc

    bf16 = mybir.dt.bfloat16
    f32 = mybir.dt.float32
    AF = mybir.ActivationFunctionType
    ALU = mybir.AluOpType
    ALL = [list(range(NCORES))]
    cht = tpc // NCHUNK

    nc = bacc.Bacc("TRN2", target_bir_lowering=False, debug=False,
                   num_devices=NCORES)

    # ---------------- external I/O ----------------
    x_sl = nc.dram_tensor("x_sl", [tpc, C], f32, kind="ExternalInput").ap()
    wa_sl = nc.dram_tensor("wa_sl", [OS, C], f32, kind="ExternalInput").ap()
    ba_sl = nc.dram_tensor("ba_sl", [OS], f32, kind="ExternalInput").ap()
    wp_sl = nc.dram_tensor("wp_sl", [C, 512], f32, kind="ExternalInput").ap()
    bp = nc.dram_tensor("bp", [C], f32, kind="ExternalInput").ap()
    mask8 = nc.dram_tensor("mask8", [128, 512], bf16, kind="ExternalInput").ap()
    ident = nc.dram_tensor("ident", [128, 128], bf16, kind="ExternalInput").ap()
    out_ext = nc.dram_tensor("out", [tpc, C], f32, kind="ExternalOutput").ap()

    # ---------------- internal DRAM ----------------
    xg_in = nc.dram_tensor("xg_in", [C, tpc], bf16).ap()
    xt_ag = [nc.dram_tensor(f"xt_ag{h}", [NCORES * C // 2, tpc], bf16,
                            addr_space="Shared").ap() for h in range(2)]
    wp_my = nc.dram_tensor("wp_my", [512, C], bf16).ap()
    wpt_ag = nc.dram_tensor("wpt_ag", [C, C], bf16, addr_space="Shared").ap()
    qkv_send = [nc.dram_tensor(f"qkv_send{i}", [NCORES, 512, tpc], bf16).ap()
                for i in range(3)]
    qkv_recv = [nc.dram_tensor(f"qkv_recv{i}", [NCORES, 512, tpc], bf16).ap()
                for i in range(3)]

    CT = C // 128        # 32 contraction tiles
    OT = OS // 128       # 12 output tiles for qkv slice
    PT = C // 128        # 32 output tiles for proj

    with tile.TileContext(nc) as tc:
        with tc.tile_pool(name="const", bufs=1) as constp:
            ba_t = constp.tile([128, OT], f32)
            nc.sync.dma_start(out=ba_t[:], in_=ba_sl.rearrange("(o p) -> p o", p=128))
            bp_t = constp.tile([128, PT], f32)
            nc.sync.dma_start(out=bp_t[:], in_=bp.rearrange("(o p) -> p o", p=128))
            mask_t = constp.tile([128, 512], bf16)
            nc.sync.dma_start(out=mask_t[:], in_=mask8[:])
            id_t = constp.tile([128, 128], bf16)
            nc.sync.dma_start(out=id_t[:], in_=ident[:])

            def pe_transpose(trps, dst_ap, src_ap):
                """128x128 bf16 transpose: PE transpose-mode + DVE evict."""
                tp = trps.tile([128, 128], bf16, tag="trp")
                nc.tensor.transpose(tp[:], src_ap, id_t[:])
                nc.vector.tensor_copy(dst_ap, tp[:])

            # ======== prep + QKV GEMM (WaT resident in SBUF) ========
            with tc.tile_pool(name="wat", bufs=1) as watp:
                wat = watp.tile([128, CT * OT * 128], bf16)

                with (
                    tc.tile_pool(name="xprep", bufs=2) as xprep,
                    tc.tile_pool(name="xts", bufs=1) as xtsp,
                    tc.tile_pool(name="wpprep", bufs=2) as wpprep,
                    tc.tile_pool(name="wpst", bufs=1) as wpstp,
                    tc.tile_pool(name="trps", bufs=4, space="PSUM") as trps,
                ):
                    # P0: x slab -> bf16 -> transpose -> AllGather (2 halves)
                    xgv = xg_in.rearrange("(c p) t -> p c t", p=128)
                    xts = xtsp.tile([128, CT, tpc], bf16)
                    for half in range(2):
                        c0 = half * (CT // 2)
                        for tt in range(tpc // 128):
                            xb = xprep.tile([128, C // 2], bf16, tag="xb")
                            nc.gpsimd.dma_start(
                                out=xb[:],
                                in_=x_sl[tt * 128:(tt + 1) * 128,
                                         c0 * 128:(c0 + CT // 2) * 128])
                            for k in range(CT // 2):
                                pe_transpose(
                                    trps,
                                    xts[:, c0 + k, tt * 128:(tt + 1) * 128],
                                    xb[:, k * 128:(k + 1) * 128])
                        nc.sync.dma_start(
                            out=xgv[:, c0:c0 + CT // 2, :],
                            in_=xts[:, c0:c0 + CT // 2, :])
                        nc.gpsimd.collective_compute(
                            "AllGather", ALU.bypass, replica_groups=ALL,
                            ins=[xg_in[c0 * 128:(c0 + CT // 2) * 128, :].opt()],
                            outs=[xt_ag[half].opt()],
                        )

                    # P1: Wa slice -> bf16 -> transpose into resident WaT
                    for ot in range(OT):
                        wab = xprep.tile([128, C], bf16, tag="xb")
                        nc.gpsimd.dma_start(
                            out=wab[:], in_=wa_sl[ot * 128:(ot + 1) * 128, :])
                        for ct in range(CT):
                            pe_transpose(
                                trps,
                                wat[:, (ct * OT + ot) * 128:
                                    (ct * OT + ot + 1) * 128],
                                wab[:, ct * 128:(ct + 1) * 128])

                    # P0b: Wp slice -> WPT slab -> AllGather
                    wpst = wpstp.tile([128, 4, C], bf16)
                    for ot in range(PT):
                        wpb = wpprep.tile([128, 512], bf16, tag="wpb")
                        nc.gpsimd.dma_start(
                            out=wpb[:], in_=wp_sl[ot * 128:(ot + 1) * 128, :])
                        for jt in range(4):
                            pe_transpose(trps,
                                         wpst[:, jt, ot * 128:(ot + 1) * 128],
                                         wpb[:, jt * 128:(jt + 1) * 128])
                    nc.sync.dma_start(
                        out=wp_my.rearrange("(j p) o -> p j o", p=128), in_=wpst[:]
                    )
                    nc.gpsimd.collective_compute(
                        "AllGather", ALU.bypass, replica_groups=ALL,
                        ins=[wp_my.opt()], outs=[wpt_ag.opt()],
                    )

                # P2: QKV GEMM (column-parallel over features, all tokens)
                with (
                    tc.tile_pool(name="xs", bufs=2) as xsp,
                    tc.tile_pool(name="qkvps", bufs=2, space="PSUM") as qkvps,
                    tc.tile_pool(name="qkvev", bufs=3) as qkvev,
                ):
                    for g in range(NCORES):
                        xs = xsp.tile([128, CT, tpc], bf16)
                        for half in range(2):
                            c0 = half * (CT // 2)
                            nc.sync.dma_start(
                                out=xs[:, c0:c0 + CT // 2, :],
                                in_=xt_ag[half][g * C // 2:(g + 1) * C // 2,
                                                :].rearrange(
                                    "(c p) t -> p c t", p=128),
                            )
                        for ot in range(OT):
                            ps = qkvps.tile([128, tpc], f32)
                            for ct in range(CT):
                                nc.tensor.matmul(
                                    ps[:],
                                    wat[:, (ct * OT + ot) * 128:
                                        (ct * OT + ot + 1) * 128],
                                    xs[:, ct, :],
                                    start=(ct == 0), stop=(ct == CT - 1),
                                )
                            ev = qkvev.tile([128, tpc], bf16)
                            nc.vector.tensor_scalar(
                                ev[:], ps[:], ba_t[:, ot:ot + 1], None, ALU.add)
                            nc.sync.dma_start(
                                out=qkv_send[ot // 4][g, (ot % 4) * 128:
                                                      (ot % 4 + 1) * 128, :],
                                in_=ev[:])

            # ======== P3: AllToAll to token-sharded (q, k, v separately) ====
            for i in range(3):
                nc.gpsimd.collective_compute(
                    "AllToAll", ALU.bypass, replica_groups=ALL,
                    ins=[qkv_send[i].opt()], outs=[qkv_recv[i].opt()],
                )

            with tc.tile_pool(name="yt", bufs=1) as ytp:
                # y^T in proj-contraction layout:
                #  partitions p<64 : (i=p)  y[t, i*64 + 2e]   at free e*tpc+t
                #  partitions p>=64: (i=p-64) y[t, i*64+2e+1] at free e*tpc+t
                yt = ytp.tile([128, 32 * tpc], bf16)
                ytv = yt.rearrange("p (e t) -> p e t", t=tpc)

                # ======== P4+P5: attention, NCHUNK chunks of tokens ========
                for ch in range(NCHUNK):
                    t0 = ch * cht
                    hh = cht // 2
                    with (
                        tc.tile_pool(name="qkt", bufs=1) as qktp,
                        tc.tile_pool(name="vt", bufs=1) as vtp,
                        tc.tile_pool(name="att_sb", bufs=4) as attsb,
                        tc.tile_pool(name="att_ps", bufs=3, space="PSUM") as attps,
                        tc.tile_pool(name="u_ps", bufs=2, space="PSUM") as ups,
                        tc.tile_pool(name="r_sb", bufs=4) as rsb,
                    ):
                        # token-halves: tm < hh -> partitions 0:64,
                        #               tm >= hh -> partitions 64:128
                        qt = qktp.tile([128, H * cht], bf16, tag="qt")
                        kt = qktp.tile([128, H * cht], bf16, tag="kt")
                        vte = vtp.tile([128, 33 * cht], bf16, tag="vte")
                        vto = vtp.tile([128, 33 * cht], bf16, tag="vto")
                        qtv = qt.rearrange("d (h t) -> d h t", t=cht)
                        ktv = kt.rearrange("d (h t) -> d h t", t=cht)
                        vtev = vte.rearrange("j (e t) -> j e t", t=cht)
                        vtov = vto.rearrange("j (e t) -> j e t", t=cht)
                        for j in range(NCORES):
                            qv = qkv_recv[0][j].rearrange(
                                "(h d) t -> d h t", d=64)
                            kv = qkv_recv[1][j].rearrange(
                                "(h d) t -> d h t", d=64)
                            vv = qkv_recv[2][j].rearrange(
                                "(h d) t -> h d t", d=64)
                            for half in range(2):
                                p0 = half * 64
                                ta = t0 + half * hh
                                nc.sync.dma_start(
                                    out=qtv[p0:p0 + 64, 8 * j:8 * (j + 1),
                                            half * hh:(half + 1) * hh],
                                    in_=qv[:, :, ta:ta + hh])
                                nc.sync.dma_start(
                                    out=ktv[p0:p0 + 64, 8 * j:8 * (j + 1),
                                            half * hh:(half + 1) * hh],
                                    in_=kv[:, :, ta:ta + hh])
                                nc.sync.dma_start(
                                    out=vtev[p0 + 8 * j:p0 + 8 * (j + 1), 0:32,
                                             half * hh:(half + 1) * hh],
                                    in_=vv[:, 0:64:2, ta:ta + hh])
                                nc.sync.dma_start(
                                    out=vtov[p0 + 8 * j:p0 + 8 * (j + 1), 0:32,
                                             half * hh:(half + 1) * hh],
                                    in_=vv[:, 1:64:2, ta:ta + hh])
                        nc.vector.memset(vte[:, 32 * cht:33 * cht], 1.0)
                        nc.vector.memset(vto[:, 32 * cht:33 * cht], 1.0)

                        # HAM warm-keepers: trivial matmuls, each gated on one
                        # shuffle DMA landing, keep PE busy-windows alive
                        # across the AllToAll/shuffle gap.
                        with tc.tile_pool(name="fill_ps", bufs=1,
                                          space="PSUM") as fillps:
                            for j in range(NCORES):
                                for src_t in (qt, kt):
                                    fp = fillps.tile([64, 64], f32, tag="f")
                                    nc.tensor.matmul(
                                        fp[:],
                                        src_t[0:64, (8 * j) * cht:
                                              (8 * j) * cht + 64],
                                        src_t[0:64, (8 * j) * cht:
                                              (8 * j) * cht + 64],
                                        start=True, stop=True)
                                    fp2 = fillps.tile([64, 64], f32, tag="f")
                                    nc.tensor.matmul(
                                        fp2[:],
                                        src_t[64:128, (8 * j + 1) * cht - 64:
                                              (8 * j + 1) * cht],
                                        src_t[64:128, (8 * j + 1) * cht - 64:
                                              (8 * j + 1) * cht],
                                        start=True, stop=True)

                        def emit_s(tb):
                            """S^T for 8 lo + 8 hi tokens, row-group pairs."""
                            psS = attps.tile([128, 512], f32, tag="psS")
                            for u in range(8):
                                tm = tb * 8 + u
                                tmh = hh + tm
                                nc.tensor.matmul(
                                    psS[0:64, u * 64:(u + 1) * 64],
                                    kt[0:64, tm:tm + (H - 1) * cht + 1:cht],
                                    qt[0:64, tm:tm + (H - 1) * cht + 1:cht],
                                    start=True, stop=True)
                                nc.tensor.matmul(
                                    psS[64:128, u * 64:(u + 1) * 64],
                                    kt[64:128, tmh:tmh + (H - 1) * cht + 1:cht],
                                    qt[64:128, tmh:tmh + (H - 1) * cht + 1:cht],
                                    start=True, stop=True)
                            e8 = attsb.tile([128, 512], bf16, tag="e8")
                            nc.scalar.activation(e8[:], psS[:], AF.Exp,
                                                 scale=0.125)
                            e8m = attsb.tile([128, 512], bf16, tag="e8m")
                            nc.vector.tensor_tensor(e8m[:], e8[:], mask_t[:],
                                                    ALU.mult)
                            return e8m

                        def emit_u(tb, e8m):
                            psU = ups.tile([128, 8 * 33], f32, tag="psUl")
                            psUh = ups.tile([128, 8 * 33], f32, tag="psUh")
                            for u in range(8):
                                tm = tb * 8 + u
                                tmh = hh + tm
                                nc.tensor.matmul(
                                    psU[0:64, u * 33:(u + 1) * 33],
                                    e8m[0:64, u * 64:(u + 1) * 64],
                                    vte[0:64, tm:tm + 32 * cht + 1:cht],
                                    start=True, stop=True)
                                nc.tensor.matmul(
                                    psUh[0:64, u * 33:(u + 1) * 33],
                                    e8m[64:128, u * 64:(u + 1) * 64],
                                    vte[64:128, tmh:tmh + 32 * cht + 1:cht],
                                    start=True, stop=True)
                                nc.tensor.matmul(
                                    psU[64:128, u * 33:(u + 1) * 33],
                                    e8m[0:64, u * 64:(u + 1) * 64],
                                    vto[0:64, tm:tm + 32 * cht + 1:cht],
                                    start=True, stop=True)
                                nc.tensor.matmul(
                                    psUh[64:128, u * 33:(u + 1) * 33],
                                    e8m[64:128, u * 64:(u + 1) * 64],
                                    vto[64:128, tmh:tmh + 32 * cht + 1:cht],
                                    start=True, stop=True)
                            for ph, pu in ((0, psU), (1, psUh)):
                                r8 = rsb.tile([128, 8], f32, tag="r8")
                                nc.vector.reciprocal(
                                    r8[:], pu[:, 32:32 + 7 * 33 + 1:33])
                                puv = pu.rearrange("p (t c) -> p c t", c=33)
                                tg0 = t0 + ph * hh + tb * 8
                                for lo, hi in ((0, 64), (64, 128)):
                                    nc.vector.tensor_tensor(
                                        ytv[lo:hi, :, tg0:tg0 + 8],
                                        puv[lo:hi, 0:32, :],
                                        r8[lo:hi, None, :].broadcast_to(
                                            [hi - lo, 32, 8]),
                                        ALU.mult)

                        pend = None
                        for tb in range(hh // 8):
                            e8m = emit_s(tb)
                            if pend is not None:
                                emit_u(tb - 1, pend)
                            pend = e8m
                        emit_u(hh // 8 - 1, pend)

                # ======== P6: proj GEMM + transpose eviction ========
                with (
                    tc.tile_pool(name="wpt_in", bufs=32) as wptin,
                    tc.tile_pool(name="projps", bufs=2, space="PSUM") as projps,
                    tc.tile_pool(name="ptrps", bufs=4, space="PSUM") as ptrps,
                    tc.tile_pool(name="projev", bufs=2) as projev,
                    tc.tile_pool(name="projtr", bufs=4) as projtr,
                ):
                    for ot in range(PT):
                        ps = projps.tile([128, tpc], f32)
                        for e4 in range(8):
                            wt4 = wptin.tile([128, 4, 128], bf16)
                            nc.sync.dma_start(
                                out=wt4[:],
                                in_=wpt_ag[e4 * 512:(e4 + 1) * 512,
                                           ot * 128:(ot + 1) * 128].rearrange(
                                               "(e p) o -> p e o", p=128))
                            for k in range(4):
                                e = e4 * 4 + k
                                nc.tensor.matmul(
                                    ps[:], wt4[:, k, :],
                                    yt[:, e * tpc:(e + 1) * tpc],
                                    start=(e == 0), stop=(e == 31),
                                )
                        ev = projev.tile([128, tpc], bf16)
                        nc.vector.tensor_scalar(
                            ev[:], ps[:], bp_t[:, ot:ot + 1], None, ALU.add)
                        for tt in range(tpc // 128):
                            tp2 = ptrps.tile([128, 128], bf16, tag="ptr")
                            nc.tensor.transpose(
                                tp2[:], ev[:, tt * 128:(tt + 1) * 128], id_t[:])
                            tr = projtr.tile([128, 128], bf16)
                            nc.vector.tensor_copy(tr[:], tp2[:])
                            nc.gpsimd.dma_start(
                                out=out_ext[tt * 128:(tt + 1) * 128,
                                            ot * 128:(ot + 1) * 128],
                                in_=tr[:])

    nc.compile()
    return nc


_CACHED_NC = None


def _get_nc():
    global _CACHED_NC
    if _CACHED_NC is None:
        _CACHED_NC = _build_program()
    return _CACHED_NC


def host_inputs(x, W_atten, b_atten, W_proj, b_proj, tpc=TPC):
    """Slice/permute the full inputs into per-core input maps."""
    x = np.asarray(x, dtype=np.float32).reshape(-1, C)
    W_atten = np.asarray(W_atten, dtype=np.float32)
    b_atten = np.asarray(b_atten, dtype=np.float32)
    W_proj = np.asarray(W_proj, dtype=np.float32)
    b_proj = np.asarray(b_proj, dtype=np.float32)

    # causal mask over head index, replicated for 8-token exp batches
    m = np.tril(np.ones((H, H), dtype=np.float32)).T  # m[j,i] = 1 if j <= i
    mask8 = np.tile(m, (2, 8)).astype(_BF16)
    identity = np.eye(128, dtype=np.float32).astype(_BF16)

    in_maps = []
    for c in range(NCORES):
        r0 = c * 512
        wa = np.concatenate([
            W_atten[r0:r0 + 512],
            W_atten[C + r0:C + r0 + 512],
            W_atten[2 * C + r0:2 * C + r0 + 512],
        ], axis=0)
        ba = np.concatenate([
            b_atten[r0:r0 + 512],
            b_atten[C + r0:C + r0 + 512],
            b_atten[2 * C + r0:2 * C + r0 + 512],
        ], axis=0)
        js = np.arange(512)
        k = js // 128
        p = js % 128
        i = p % 64
        d = 8 * c + 2 * k + (p >= 64)
        cols = i * 64 + d
        wp = np.ascontiguousarray(W_proj[:, cols])
        in_maps.append({
            "x_sl": np.ascontiguousarray(x[c * tpc:(c + 1) * tpc]),
            "wa_sl": np.ascontiguousarray(wa),
            "ba_sl": np.ascontiguousarray(ba),
            "wp_sl": wp,
            "bp": b_proj,
            "mask8": mask8,
            "ident": identity,
        })
    return in_maps


def kernel(x, W_atten, b_atten, W_proj, b_proj):
    from concourse.bass_utils import run_bass_kernel_spmd

    nc = _get_nc()
    in_maps = host_inputs(x, W_atten, b_atten, W_proj, b_proj)
    res = run_bass_kernel_spmd(nc, in_maps, list(range(NCORES)))
    out = np.concatenate([res.results[c]["out"] for c in range(NCORES)], axis=0)
    return out.reshape(B, T, C)
